# revision 11
# baseline (speedup 1.0000x reference)
"""LocalCrossAttention Trainium2 kernel (8-core SPMD).

Math refactoring (exact up to fp associativity):
  scores1 = q2 @ k1.T with q2 = x2 Wq2^T + bq2, k1 = x1 Wk1^T + bk1.
  q2 @ k1.T = (x2 Wq2^T + bq2) Wk1 x1^T + (q2 . bk1)[row-const]
  The row-constant term drops inside softmax, so bk is never needed and
  the full k projection never has to be materialized:
      S = ((x_q W_q^T + b_q) @ W_k) @ x_kv^T * scale
  Likewise rows of P sum to 1, so the v bias factors out:
      ctx = P @ (x_kv W_v^T + b_v) = (P @ x_kv) @ W_v^T + b_v
  Each core therefore only needs its 512-row query shard plus the raw
  (unprojected) opposite-stream activations => no replicated projection
  FLOPs, no collectives; per-core FLOPs = total/8.

Sharding: core c computes rows [c*512,(c+1)*512) of context1 (queries
from x2) and of context2 (queries from x1). Host concatenates.
"""

import contextlib
import os

import numpy as np

import concourse.bass as bass
import concourse.bacc as bacc
import concourse.mybir as mybir
import concourse.tile as tile
from concourse.bass_utils import run_bass_kernel_spmd
from concourse.masks import make_identity

N = 4096
D = 1024
P = 128
NCORES = 8
SH = N // NCORES          # 512 query rows per core
DC = D // P               # 8 feature chunks
ICH = SH // P             # 4 query-row chunks
JB = 512                  # kv block size
NJB = N // JB             # 8 kv blocks
JS = JB // P              # 4 sub-blocks per kv block
SCALE = 1.0 / float(np.sqrt(D))

F32 = mybir.dt.float32
F32R = mybir.dt.float32r
AF = mybir.ActivationFunctionType
AX = mybir.AxisListType

# matmul dtype mode: "f32r" (1 cyc/row, fp32 bits through fast path) or "f32"
MM_MODE = os.environ.get("XATTN_MM_MODE", "f32r")


def _mm(ap):
    return ap.bitcast(F32R) if MM_MODE == "f32r" else ap


def _ap(x):
    return x if isinstance(x, bass.AP) else x.ap()


def _emit_stream(es, tc, nc, ident, ps_mm, ps_tr, xqs_d, wq_d, bq_d, wk_d,
                 wv_d, bv_d, xkv_d, out_d, tag):
    """Emit one cross-attention stream. xqs_d: [SH,D] query-side shard,
    xkv_d: [N,D] full opposite stream, out_d: [SH,D]."""
    t = tag
    cpool = es.enter_context(tc.tile_pool(name=f"const{t}", bufs=1))

    bq_sb = cpool.tile([P, DC], F32, name=f"bq{t}")
    nc.sync.dma_start(bq_sb, _ap(bq_d).rearrange("(c p) -> p c", p=P))
    negmax = cpool.tile([P, ICH], F32, name=f"negmax{t}")
    rowsum = cpool.tile([P, ICH], F32, name=f"rowsum{t}")
    recip = cpool.tile([P, ICH], F32, name=f"recip{t}")

    spool = es.enter_context(tc.tile_pool(name=f"stream{t}", bufs=1))
    u1T = spool.tile([P, DC, SH], F32, name=f"u1T{t}")      # [d, i] 16KB/p
    c1T = spool.tile([P, DC, SH], F32, name=f"c1T{t}")      # [e, i] 16KB/p
    if True:
        # ---- Phase A: q = xq Wq^T + bq (chunked, Wq transposed on the
        # fly through PE); u1T = Wk^T-contraction of q; scale folded in.
        with contextlib.ExitStack() as ea:
            a2 = ea.enter_context(tc.tile_pool(name=f"pA2{t}", bufs=1))
            wk_nat = a2.tile([P, DC, D], F32, name=f"wkn{t}")   # [o, d]
            nc.sync.dma_start(wk_nat,
                              _ap(wk_d).rearrange("(c p) d -> p c d", p=P))
            wk_r = a2.tile([P, DC, D], F32, name=f"wkr{t}")
            nc.any.tensor_copy(_mm(wk_r), wk_nat)
            qT = a2.tile([P, DC, SH], F32, name=f"qT{t}")       # [o, i]

            with contextlib.ExitStack() as ea1:
                a1 = ea1.enter_context(tc.tile_pool(name=f"pA1{t}", bufs=1))
                xq_nat = a1.tile([P, ICH, D], F32, name=f"xqn{t}")  # [i, d]
                nc.sync.dma_start(
                    xq_nat, _ap(xqs_d).rearrange("(c p) d -> p c d", p=P))
                xqT = a1.tile([P, DC, SH], F32, name=f"xqT{t}")     # [d, i]
                for dc in range(DC):
                    ps = ps_tr.tile([P, 512], F32, name=f"pst{t}", tag="tr")
                    for ii in range(ICH):
                        nc.tensor.transpose(
                            ps[:, ii * P:(ii + 1) * P],
                            xq_nat[:, ii, dc * P:(dc + 1) * P], ident)
                    nc.any.tensor_copy(_mm(xqT[:, dc, :]), ps)

                for oh in range(2):          # Wq in two 512-row halves
                    wqh = a1.tile([P, 4, D], F32, name=f"wqh{t}",
                                  tag=f"wqh{t}", bufs=2)
                    nc.sync.dma_start(
                        wqh, _ap(wq_d)[oh * 512:(oh + 1) * 512, :]
                        .rearrange("(c p) d -> p c d", p=P))
                    for o4 in range(4):
                        oc = oh * 4 + o4
                        # wqt[:, dc, :] = Wq[oc-chunk, dc-chunk].T
                        wqt = a1.tile([P, DC, P], F32, name=f"wqt{t}",
                                      tag=f"wqt{t}", bufs=2)
                        for g in range(2):
                            ps = ps_tr.tile([P, 512], F32, name=f"pst{t}",
                                            tag="tr")
                            for k in range(4):
                                dc = g * 4 + k
                                nc.tensor.transpose(
                                    ps[:, k * P:(k + 1) * P],
                                    wqh[:, o4, dc * P:(dc + 1) * P], ident)
                            nc.any.tensor_copy(
                                _mm(wqt[:, g * 4:(g + 1) * 4, :]), ps)
                        ps = ps_mm.tile([P, 512], F32, name=f"psm{t}",
                                        tag="mm")
                        for dc in range(DC):
                            nc.tensor.matmul(ps, _mm(wqt[:, dc, :]),
                                             _mm(xqT[:, dc, :]),
                                             start=(dc == 0),
                                             stop=(dc == DC - 1))
                        nc.scalar.activation(_mm(qT[:, oc, :]), ps,
                                             AF.Identity,
                                             bias=bq_sb[:, oc:oc + 1])

            for dc in range(DC):
                ps = ps_mm.tile([P, 512], F32, name=f"psm{t}", tag="mm")
                for oc in range(DC):
                    nc.tensor.matmul(ps,
                                     _mm(wk_r[:, oc, dc * P:(dc + 1) * P]),
                                     _mm(qT[:, oc, :]),
                                     start=(oc == 0), stop=(oc == DC - 1))
                nc.scalar.activation(_mm(u1T[:, dc, :]), ps, AF.Copy,
                                     scale=SCALE)

        with contextlib.ExitStack() as e_s:
            sp = e_s.enter_context(tc.tile_pool(name=f"pS{t}", bufs=1))
            S = sp.tile([P, ICH, N], F32, name=f"S{t}")     # [i, j] 64KB/p

            # ---- Phase B: S = u1T.T @ xkv^T over kv blocks ----
            with contextlib.ExitStack() as eb:
                bpool = eb.enter_context(tc.tile_pool(name=f"pB{t}", bufs=1))
                for jb in range(NJB):
                    xb = bpool.tile([P, JS, D], F32, name=f"xb{t}",
                                    tag=f"xb{t}", bufs=2)
                    nc.sync.dma_start(
                        xb, _ap(xkv_d)[jb * JB:(jb + 1) * JB, :]
                        .rearrange("(c p) d -> p c d", p=P))
                    xbT = bpool.tile([P, DC, JB], F32, name=f"xbT{t}",
                                     tag=f"xbT{t}", bufs=2)
                    for dc in range(DC):
                        ps = ps_tr.tile([P, 512], F32, name=f"pst{t}",
                                        tag="tr")
                        for js in range(JS):
                            nc.tensor.transpose(
                                ps[:, js * P:(js + 1) * P],
                                xb[:, js, dc * P:(dc + 1) * P], ident)
                        nc.any.tensor_copy(_mm(xbT[:, dc, :]), ps)
                    for ic in range(ICH):
                        ps = ps_mm.tile([P, 512], F32, name=f"psm{t}",
                                        tag="mm")
                        for dc in range(DC):
                            nc.tensor.matmul(
                                ps, _mm(u1T[:, dc, ic * P:(ic + 1) * P]),
                                _mm(xbT[:, dc, :]),
                                start=(dc == 0), stop=(dc == DC - 1))
                        nc.any.tensor_copy(
                            S[:, ic, jb * JB:(jb + 1) * JB], ps)

            # u1T no longer needed -> e_u closes via enclosing scope order
            # ---- Phase C: softmax rows (normalization deferred) ----
            for ic in range(ICH):
                nc.vector.reduce_max(negmax[:, ic:ic + 1], S[:, ic, :],
                                     axis=AX.X, negate=True)
                nc.scalar.activation(S[:, ic, :], S[:, ic, :], AF.Exp,
                                     bias=negmax[:, ic:ic + 1], scale=1.0,
                                     accum_out=rowsum[:, ic:ic + 1])
                nc.vector.reciprocal(recip[:, ic:ic + 1],
                                     rowsum[:, ic:ic + 1])

            # ---- Phase D: c1T[e,i] = sum_j xkv[j,e] P[i,j] ----
            with contextlib.ExitStack() as ed:
                dpool = ed.enter_context(tc.tile_pool(name=f"pD{t}", bufs=1))
                for jb in range(NJB):
                    xb = dpool.tile([P, JS, D], F32, name=f"xb2{t}",
                                    tag=f"xb2{t}", bufs=2)
                    nc.sync.dma_start(
                        xb, _ap(xkv_d)[jb * JB:(jb + 1) * JB, :]
                        .rearrange("(c p) d -> p c d", p=P))
                    xbr = dpool.tile([P, JS, D], F32, name=f"xbr{t}",
                                     tag=f"xbr{t}", bufs=2)
                    nc.any.tensor_copy(_mm(xbr), xb)
                    pT = dpool.tile([P, JS, SH], F32, name=f"pT{t}",
                                    tag=f"pT{t}", bufs=2)
                    for js in range(JS):
                        ps = ps_tr.tile([P, 512], F32, name=f"pst{t}",
                                        tag="tr")
                        for ic in range(ICH):
                            nc.tensor.transpose(
                                ps[:, ic * P:(ic + 1) * P],
                                S[:, ic,
                                  jb * JB + js * P: jb * JB + (js + 1) * P],
                                ident)
                        nc.any.tensor_copy(_mm(pT[:, js, :]), ps)
                    for ec in range(DC):
                        ps = ps_mm.tile([P, 512], F32, name=f"psm{t}",
                                        tag="mm")
                        for js in range(JS):
                            nc.tensor.matmul(
                                ps, _mm(xbr[:, js, ec * P:(ec + 1) * P]),
                                _mm(pT[:, js, :]),
                                start=(js == 0), stop=(js == JS - 1))
                        if jb == 0:
                            nc.any.tensor_copy(_mm(c1T[:, ec, :]), ps)
                        else:
                            nc.vector.tensor_add(_mm(c1T[:, ec, :]),
                                                 c1T[:, ec, :], ps)

    # ---- Phase E: ctx = (c1 @ Wv^T) * recip + bv ----
    with contextlib.ExitStack() as ee:
        epool = ee.enter_context(tc.tile_pool(name=f"pE{t}", bufs=1))
        bv_sb = epool.tile([1, D], F32, name=f"bv{t}")
        nc.sync.dma_start(bv_sb, _ap(bv_d)[None, :])
        ones1 = epool.tile([1, P], F32, name=f"ones{t}")
        nc.vector.memset(ones1, 1.0)
        bv_bc = epool.tile([P, D], F32, name=f"bvbc{t}")
        for h in range(2):
            ps = ps_mm.tile([P, 512], F32, name=f"psm{t}", tag="mm")
            nc.tensor.matmul(ps, ones1, bv_sb[0:1, h * 512:(h + 1) * 512],
                             start=True, stop=True)
            nc.any.tensor_copy(bv_bc[:, h * 512:(h + 1) * 512], ps)
        wv_nat = epool.tile([P, DC, D], F32, name=f"wvn{t}")   # [o, e]
        nc.sync.dma_start(wv_nat,
                          _ap(wv_d).rearrange("(c p) d -> p c d", p=P))
        wvT = epool.tile([P, DC, D], F32, name=f"wvT{t}")      # [e, o]
        for ec in range(DC):
            for og in range(0, DC, 4):
                ps = ps_tr.tile([P, 512], F32, name=f"pst{t}", tag="tr")
                for oo in range(4):
                    nc.tensor.transpose(
                        ps[:, oo * P:(oo + 1) * P],
                        wv_nat[:, og + oo, ec * P:(ec + 1) * P], ident)
                nc.any.tensor_copy(_mm(wvT[:, ec, og * P:(og + 4) * P]), ps)

        for ic in range(ICH):
            ctx_sb = epool.tile([P, D], F32, name=f"ctx{t}", tag=f"ctx{t}",
                                bufs=2)
            for oh in range(2):
                ps = ps_mm.tile([P, 512], F32, name=f"psm{t}", tag="mm")
                for ec in range(DC):
                    nc.tensor.matmul(ps, _mm(c1T[:, ec, ic * P:(ic + 1) * P]),
                                     _mm(wvT[:, ec, oh * 512:(oh + 1) * 512]),
                                     start=(ec == 0), stop=(ec == DC - 1))
                nc.scalar.activation(ctx_sb[:, oh * 512:(oh + 1) * 512], ps,
                                     AF.Copy, scale=recip[:, ic:ic + 1])
                nc.vector.tensor_add(
                    ctx_sb[:, oh * 512:(oh + 1) * 512],
                    ctx_sb[:, oh * 512:(oh + 1) * 512],
                    bv_bc[:, oh * 512:(oh + 1) * 512])
            nc.sync.dma_start(_ap(out_d)[ic * P:(ic + 1) * P, :], ctx_sb)


def build():
    nc = bacc.Bacc("TRN2", target_bir_lowering=False, debug=False,
                   num_devices=NCORES)
    d = {}
    for name, shape in [("x1", (N, D)), ("x2", (N, D)),
                        ("x1s", (SH, D)), ("x2s", (SH, D))]:
        d[name] = nc.dram_tensor(name, shape, F32, kind="ExternalInput")
    for s in ("1", "2"):
        for w in ("wq", "wk", "wv"):
            d[w + s] = nc.dram_tensor(w + s, (D, D), F32, kind="ExternalInput")
        for b in ("bq", "bv"):
            d[b + s] = nc.dram_tensor(b + s, (D,), F32, kind="ExternalInput")
    d["ctx1s"] = nc.dram_tensor("ctx1s", (SH, D), F32, kind="ExternalOutput")
    d["ctx2s"] = nc.dram_tensor("ctx2s", (SH, D), F32, kind="ExternalOutput")

    with tile.TileContext(nc) as tc, contextlib.ExitStack() as es:
        gpool = es.enter_context(tc.tile_pool(name="g", bufs=1))
        ident = gpool.tile([P, P], F32, name="ident")
        make_identity(nc, ident)
        ps_mm = es.enter_context(tc.tile_pool(name="psmm", bufs=4,
                                              space="PSUM"))
        ps_tr = es.enter_context(tc.tile_pool(name="pstr", bufs=4,
                                              space="PSUM"))
        # stream 1: queries from x2 shard, kv side from full x1
        with contextlib.ExitStack() as es_a:
            _emit_stream(es_a, tc, nc, ident, ps_mm, ps_tr, d["x2s"],
                         d["wq2"], d["bq2"], d["wk1"], d["wv1"], d["bv1"],
                         d["x1"], d["ctx1s"], "a")
        # stream 2: queries from x1 shard, kv side from full x2
        with contextlib.ExitStack() as es_b:
            _emit_stream(es_b, tc, nc, ident, ps_mm, ps_tr, d["x1s"],
                         d["wq1"], d["bq1"], d["wk2"], d["wv2"], d["bv2"],
                         d["x2"], d["ctx2s"], "b")
    nc.compile()
    return nc


_NC_CACHE = None


def _get_nc():
    global _NC_CACHE
    if _NC_CACHE is None:
        _NC_CACHE = build()
    return _NC_CACHE


def _in_maps(inputs):
    f = lambda x: np.ascontiguousarray(np.asarray(x), dtype=np.float32)
    x1, x2 = f(inputs["input_tensor1"]), f(inputs["input_tensor2"])
    base = {
        "x1": x1, "x2": x2,
        "wq1": f(inputs["Wq1"]), "bq1": f(inputs["bq1"]),
        "wk1": f(inputs["Wk1"]),
        "wv1": f(inputs["Wv1"]), "bv1": f(inputs["bv1"]),
        "wq2": f(inputs["Wq2"]), "bq2": f(inputs["bq2"]),
        "wk2": f(inputs["Wk2"]),
        "wv2": f(inputs["Wv2"]), "bv2": f(inputs["bv2"]),
    }
    maps = []
    for c in range(NCORES):
        m = dict(base)
        m["x1s"] = np.ascontiguousarray(x1[c * SH:(c + 1) * SH])
        m["x2s"] = np.ascontiguousarray(x2[c * SH:(c + 1) * SH])
        maps.append(m)
    return maps


def run(inputs, trace=False):
    nc = _get_nc()
    res = run_bass_kernel_spmd(nc, _in_maps(inputs), list(range(NCORES)),
                               trace=trace)
    ctx1 = np.concatenate([res.results[c]["ctx1s"] for c in range(NCORES)], 0)
    ctx2 = np.concatenate([res.results[c]["ctx2s"] for c in range(NCORES)], 0)
    return (ctx1, ctx2), res


def kernel(**inputs):
    out, _ = run(inputs, trace=False)
    return out


def build_timing(reps=4):
    """Timing-only build: inputs generated on device (zeros), two-stream
    computation unrolled `reps` times, tiny external I/O so per-call
    transfer overhead stays at the dispatch floor."""
    nc = bacc.Bacc("TRN2", target_bir_lowering=False, debug=False,
                   num_devices=NCORES)
    dummy = nc.dram_tensor("tdum", (P, P), F32, kind="ExternalInput")
    tout = nc.dram_tensor("tout", (P, P), F32, kind="ExternalOutput")

    with tile.TileContext(nc) as tc, contextlib.ExitStack() as es:
        dram = es.enter_context(tc.tile_pool(name="dram", bufs=1,
                                             space="DRAM"))
        x1d = dram.tile([N, D], F32, name="x1d")
        wd = dram.tile([D, D], F32, name="wd")
        bd = dram.tile([D], F32, name="bd")
        ctx1d = dram.tile([SH, D], F32, name="ctx1d")
        ctx2d = dram.tile([SH, D], F32, name="ctx2d")

        gpool = es.enter_context(tc.tile_pool(name="g", bufs=1))
        ident = gpool.tile([P, P], F32, name="ident")
        make_identity(nc, ident)
        zs = gpool.tile([P, D], F32, name="zs")
        nc.vector.memset(zs, 0.0)
        for ch in range(N // P):
            nc.sync.dma_start(x1d[ch * P:(ch + 1) * P, :], zs)
        for ch in range(D // P):
            nc.sync.dma_start(wd[ch * P:(ch + 1) * P, :], zs)
        nc.sync.dma_start(bd[None, :], zs[0:1, :])

        ps_mm = es.enter_context(tc.tile_pool(name="psmm", bufs=4,
                                              space="PSUM"))
        ps_tr = es.enter_context(tc.tile_pool(name="pstr", bufs=4,
                                              space="PSUM"))
        for r in range(reps):
            with contextlib.ExitStack() as es_a:
                _emit_stream(es_a, tc, nc, ident, ps_mm, ps_tr,
                             x1d[0:SH, :], wd, bd, wd, wd, bd, x1d, ctx1d,
                             f"a{r}")
            with contextlib.ExitStack() as es_b:
                _emit_stream(es_b, tc, nc, ident, ps_mm, ps_tr,
                             x1d[SH:2 * SH, :], wd, bd, wd, wd, bd, x1d,
                             ctx2d, f"b{r}")
        with tc.tile_pool(name="fin", bufs=1) as fin:
            ft = fin.tile([P, P], F32, name="ft")
            nc.sync.dma_start(ft, ctx1d[0:P, 0:P])
            nc.sync.dma_start(tout.ap(), ft)
    nc.compile()
    return nc
